# revision 45
# baseline (speedup 1.0000x reference)
"""Trainium2 Bass kernel for nn_MHA_48120813584614 (dual cross-attention MHA).

Strategy (head/tensor parallel over 8 cores):
  - Core c owns head c for BOTH attention directions:
      pair 0 ("i"): metadata queries attend image keys/values  -> contributes to out_i
      pair 1 ("m"): image queries attend metadata keys/values -> contributes to out_m
  - All tensors kept transposed ([feature, token]) so no transposes are needed:
      QT[e,m] = WqT.T @ XTq ; KT[e,n] = WkT.T @ XTkv ; V[n,e] = XTkv.T @ WvT
      ST[n,m] = KT.T @ QT   (scores transposed: keys on partitions)
      E = exp(ST/sqrt(512)) (no max subtraction; logits are O(1) here)
      colsum[m] = ones.T @ E (softmax denominator via PE)
      OT[e,m]  = V.T @ E     (unnormalized attention output)
      partialT[o,m] = WlT.T @ OT ; then scaled by 1/colsum[m] (normalization
      commutes with the head-output linear)
  - Row-parallel output linear: per-head partials are ReduceScattered over the
    8 cores in per-mc chunks (each RS overlaps subsequent compute; only the
    last chunk's RS is exposed). Each core gets a 64-row shard of the
    transposed output, adds b_lin slice + residual slice, and returns it.
    Host concatenates + transposes.
  - All matmuls fp8e4 with MatmulPerfMode.DoubleRow (two 128-row K-tiles per
    instruction, 2x bf16 throughput; fp32 PSUM accumulation). Host pre-scales
    Wq/Wk/Wl by 8 to lift fp8 subnormals; exp carries a -ln(4) bias so the PV
    accumulation fits fp8 range (both factors cancel via the deferred
    softmax normalization).
  - Software-pipelined schedule: scores(mc+1) is emitted before out_phase(mc)
    so the in-order PE never waits on the ACT exp drains (which would also
    drop the PE out of its 2.4GHz boost p-state); pair 1's projection matmuls
    are interleaved into pair 0's attention blocks as filler work.
  - Input loads ride the ACT hardware DMA queue, stores the SP queue, so a
    rep's prologue loads are not serialized behind the previous epilogue.
"""

import sys

sys.path.insert(0, "/opt/trn_rl_repo")

import math

import ml_dtypes
import numpy as np

import concourse.bass as bass
import concourse.mybir as mybir
import concourse.tile as tile
from concourse import bacc
from concourse.bass_utils import run_bass_kernel_spmd

H = 8
D = 512
N = 2048
NCORES = 8
P = 128
MC = 512  # m-chunk (matmul free dim / PSUM bank)
NMC = N // MC  # 4
ET = D // P  # 4 e/d/o tiles
NT = N // P  # 16 n tiles
WS = 8.0  # host-side scale on Wq/bq, Wk/bk, Wl (lifts fp8 subnormals)
SCALE = 1.0 / math.sqrt(D) / (WS * WS)  # undoes Wq*Wk scaling in the exp
# E is computed as exp(logits)/4 so the PV accumulation (max |OT| ~ 350
# unscaled) fits fp8e4's +-240 range; the factor cancels between the
# numerator (OT) and the softmax denominator (colsum).
EXP_BIAS = -math.log(4.0)

bf16 = mybir.dt.bfloat16
f32 = mybir.dt.float32
fp8 = mybir.dt.float8e4

AF = mybir.ActivationFunctionType
DR = mybir.MatmulPerfMode.DoubleRow


def _build(reps=1, single=False, no_cc=False, dump=False):
    ndev = 1 if single else NCORES
    nc = bacc.Bacc("TRN2", target_bir_lowering=False, debug=False, num_devices=ndev)

    def din(name, shape, dtype):
        return nc.dram_tensor(name, shape, dtype, kind="ExternalInput").ap()

    dumps = {}
    if dump:
        for nm, shape in [("d_qt", [P, ET, N]), ("d_kt", [P, ET, N]),
                          ("d_v", [P, NT, D]), ("d_e", [P, NT, MC]),
                          ("d_ot", [P, ET, MC]), ("d_x", [P, ET, N])]:
            dumps[nm] = nc.dram_tensor(nm, shape, f32, kind="ExternalOutput").ap()

    xt = [din("xt_i", [D, N], fp8), din("xt_m", [D, N], fp8)]
    wq = [din(f"wq{p}", [D, D], fp8) for p in range(2)]
    wk = [din(f"wk{p}", [D, D], fp8) for p in range(2)]
    wv = [din(f"wv{p}", [D, D], fp8) for p in range(2)]
    wl = [din(f"wl{p}", [D, D], fp8) for p in range(2)]
    bq = [din(f"bq{p}", [P, ET], f32) for p in range(2)]
    bv = [din(f"bv{p}", [1, D], f32) for p in range(2)]
    resid = [din("resid_i", [64, N], f32), din("resid_m", [64, N], f32)]
    out_d = [
        nc.dram_tensor("out_i", [64, N], f32, kind="ExternalOutput").ap(),
        nc.dram_tensor("out_m", [64, N], f32, kind="ExternalOutput").ap(),
    ]

    with tile.TileContext(nc) as tc:
        with (
            tc.tile_pool(name="const", bufs=2) as cpool,
            tc.tile_pool(name="xt", bufs=1) as xtpool,
            tc.tile_pool(name="w", bufs=2) as wpool,
            tc.tile_pool(name="qkv", bufs=2) as qkvpool,
            tc.tile_pool(name="v", bufs=2) as vpool,
            tc.tile_pool(name="expst", bufs=2) as epool,
            tc.tile_pool(name="ot", bufs=2) as otpool,
            tc.tile_pool(name="small", bufs=2) as spool,
            tc.tile_pool(name="outsb", bufs=2) as opool,
            tc.tile_pool(name="post", bufs=2) as ppool,
            tc.tile_pool(name="psum", bufs=6, space="PSUM") as ps,
            tc.tile_pool(name="pscs", bufs=2, space="PSUM") as pscs,
            tc.tile_pool(name="dram", bufs=1, space="DRAM") as dr,
        ):
          for _rep in range(reps):
            # ---- weights: pair 0 first (the first matmuls need wq0) ----
            def load_w(p):
                d = {}
                for nm, w_d in (("wq", wq[p]), ("wk", wk[p]), ("wv", wv[p]),
                                ("wl", wl[p])):
                    t = wpool.tile([P, ET, D], fp8, tag=nm, name=nm)
                    nc.scalar.dma_start(
                        t[:], w_d.rearrange("(t p) e -> p t e", p=P))
                    d[nm] = t
                d["bq"] = wpool.tile([P, ET], f32, tag="bq", name="bq")
                nc.scalar.dma_start(d["bq"][:], bq[p][:])
                d["bv"] = wpool.tile([1, D], f32, tag="bv", name="bv")
                nc.scalar.dma_start(d["bv"][:], bv[p][:])
                return d

            # DMA order tuned so the first QT matmul (wq0 x xt_m) can start
            # as early as possible: wq0/bq0, xt_m, wk0, xt_i, the rest.
            xts = [xtpool.tile([P, ET, N], fp8, tag=f"xt{i}", name=f"xt{i}")
                   for i in range(2)]
            w0 = {"wq": None}
            w0["wq"] = wpool.tile([P, ET, D], fp8, tag="wq", name="wq")
            nc.scalar.dma_start(w0["wq"][:],
                                wq[0].rearrange("(t p) e -> p t e", p=P))
            w0["bq"] = wpool.tile([P, ET], f32, tag="bq", name="bq")
            nc.scalar.dma_start(w0["bq"][:], bq[0][:])
            nc.scalar.dma_start(
                xts[1][:], xt[1].rearrange("(t p) n -> p t n", p=P))
            w0["wk"] = wpool.tile([P, ET, D], fp8, tag="wk", name="wk")
            nc.scalar.dma_start(w0["wk"][:],
                                wk[0].rearrange("(t p) e -> p t e", p=P))
            nc.scalar.dma_start(
                xts[0][:], xt[0].rearrange("(t p) n -> p t n", p=P))
            for nm, w_d in (("wv", wv[0]), ("wl", wl[0])):
                w0[nm] = wpool.tile([P, ET, D], fp8, tag=nm, name=nm)
                nc.scalar.dma_start(w0[nm][:],
                                    w_d.rearrange("(t p) e -> p t e", p=P))
            w0["bv"] = wpool.tile([1, D], f32, tag="bv", name="bv")
            nc.scalar.dma_start(w0["bv"][:], bv[0][:])
            wts = [w0, load_w(1)]
            # WS folded into the colsum: cs = WS*sum(E) divides out the
            # WS-scaled linear partials.
            # 128 identical columns (dual-fp8 LdWeights rejects narrow
            # stationaries); every PSUM row gets the same colsum.
            ones8 = cpool.tile([P, 2, P], fp8, tag="ones8")
            nc.any.memset(ones8[:], WS)
            expbias = cpool.tile([P, 1], f32, tag="expbias")
            nc.any.memset(expbias[:], EXP_BIAS)
            resid_sb = []
            for i in range(2):
                rt = cpool.tile([64, N], f32, tag=f"resid{i}")
                nc.scalar.dma_start(rt[:], resid[i][:])
                resid_sb.append(rt)

            # per-mc reduce-scatter chunks: only the last one is exposed
            rs_in = [[dr.tile([D, MC], fp8, tag=f"rsin{p}{mc}",
                              name=f"rsin{p}{mc}") for mc in range(NMC)]
                     for p in range(2)]
            rs_out = [[dr.tile([64, MC], fp8, tag=f"rsout{p}{mc}",
                               name=f"rsout{p}{mc}") for mc in range(NMC)]
                      for p in range(2)]

            # projection outputs for both pairs (pair 1's projections are
            # interleaved into pair 0's attention to fill PE stalls)
            proj = []
            for p in range(2):
                proj.append({
                    "qt": qkvpool.tile([P, ET, N], fp8, tag="qt", name="qt"),
                    "kt": qkvpool.tile([P, ET, N], fp8, tag="kt", name="kt"),
                    "v": vpool.tile([P, NT, D], fp8, tag="v", name="v"),
                })
            bv_bcs = []
            for p in range(2):
                t = wpool.tile([P, D], f32, tag="bvbc", name="bvbc")
                nc.gpsimd.partition_broadcast(t[:], wts[p]["bv"][:])
                bv_bcs.append(t)

            def proj_units(p):
                """Emission closures: QT/KT per (eb, mc), V per nt.
                k carries no bias: a per-head constant added to every key
                shifts all logits of a query equally and cancels in softmax."""
                xq, xkv = xts[1 - p], xts[p]
                qt_t, kt_t, v_t = proj[p]["qt"], proj[p]["kt"], proj[p]["v"]
                wq_t, wk_t, wv_t = wts[p]["wq"], wts[p]["wk"], wts[p]["wv"]
                bq_t = wts[p]["bq"]

                def qk_unit(w_p, x_p, dst_p, b_p, eb, mc):
                    def emit():
                        psq = ps.tile([P, MC], f32, tag="ps", name="psq")
                        for dt_ in range(ET // 2):
                            nc.tensor.matmul(
                                psq[:],
                                w_p[:, 2 * dt_:2 * dt_ + 2, eb * P:(eb + 1) * P],
                                x_p[:, 2 * dt_:2 * dt_ + 2, mc * MC:(mc + 1) * MC],
                                start=(dt_ == 0),
                                stop=(dt_ == ET // 2 - 1),
                                perf_mode=DR,
                            )
                        if b_p is not None:
                            nc.scalar.activation(
                                dst_p[:, eb, mc * MC:(mc + 1) * MC], psq[:],
                                AF.Identity, bias=b_p[:, eb:eb + 1],
                            )
                        else:
                            # Pool/GPSIMD cannot read PSUM on HW: DVE
                            nc.vector.tensor_copy(
                                dst_p[:, eb, mc * MC:(mc + 1) * MC], psq[:])
                    return emit

                def v_unit(nt):
                    def emit():
                        psv = ps.tile([P, D], f32, tag="ps", name="psv")
                        for dt_ in range(ET // 2):
                            nc.tensor.matmul(
                                psv[:],
                                xkv[:, 2 * dt_:2 * dt_ + 2, nt * P:(nt + 1) * P],
                                wv_t[:, 2 * dt_:2 * dt_ + 2, :],
                                start=(dt_ == 0),
                                stop=(dt_ == ET // 2 - 1),
                                perf_mode=DR,
                            )
                        nc.vector.tensor_tensor(
                            v_t[:, nt, :], psv[:], bv_bcs[p][:],
                            mybir.AluOpType.add)
                    return emit

                units = []
                for eb in range(ET):
                    for mc in range(NMC):
                        units.append(qk_unit(wq_t, xq, qt_t, bq_t, eb, mc))
                for eb in range(ET):
                    for mc in range(NMC):
                        units.append(qk_unit(wk_t, xkv, kt_t, None, eb, mc))
                for nt in range(NT):
                    units.append(v_unit(nt))
                return units

            # deferred per-chunk epilogue work, flushed ~2 blocks after its
            # reduce-scatter was issued (so the RS is long done)
            po_queue = []

            def po_chunk(p, mc):
                def emit():
                    po_bf = ppool.tile([64, MC], fp8, tag="pobf", name="pobf")
                    src = (rs_out[p][mc][:] if not (single or no_cc)
                           else rs_in[p][mc][0:64, :])
                    nc.sync.dma_start(po_bf[:], src)
                    po = ppool.tile([64, MC], f32, tag="po", name="po")
                    nc.vector.tensor_tensor(
                        po[:], po_bf[:], resid_sb[p][:, mc * MC:(mc + 1) * MC],
                        mybir.AluOpType.add,
                    )
                    nc.sync.dma_start(out_d[p][:, mc * MC:(mc + 1) * MC], po[:])
                return emit

            def scores_phase(p, mc):
                qt_t, kt_t = proj[p]["qt"], proj[p]["kt"]
                e_t = epool.tile([P, NT, MC], fp8, tag="e", name="e")
                for nt in range(NT):
                    pss = ps.tile([P, MC], f32, tag="ps", name="pss")
                    for eb in range(ET // 2):
                        nc.tensor.matmul(
                            pss[:],
                            kt_t[:, 2 * eb:2 * eb + 2, nt * P:(nt + 1) * P],
                            qt_t[:, 2 * eb:2 * eb + 2, mc * MC:(mc + 1) * MC],
                            start=(eb == 0),
                            stop=(eb == ET // 2 - 1),
                            perf_mode=DR,
                        )
                    nc.scalar.activation(e_t[:, nt, :], pss[:], AF.Exp,
                                         scale=SCALE, bias=expbias[:])
                return e_t

            def out_phase(p, mc, e_t):
                v_t, wl_t = proj[p]["v"], wts[p]["wl"]
                # softmax denominator on PE: one stationary (128 identical
                # ones columns -> every PSUM row holds the colsum, so the
                # reciprocal needs no partition broadcast). Emitted a full
                # scores-phase after e_t completed: no exp-wait stall.
                cs = pscs.tile([P, MC], f32, tag="cs", name="cs")
                for j in range(NT // 2):
                    nc.tensor.matmul(
                        cs[:],
                        ones8[:],
                        e_t[:, 2 * j:2 * j + 2, :],
                        start=(j == 0),
                        stop=(j == NT // 2 - 1),
                        perf_mode=DR,
                    )
                rb = spool.tile([P, MC], f32, tag="rb", name="rb")
                nc.vector.reciprocal(rb[:], cs[:])
                # PV: OT[e, m] unnormalized
                ot_t = otpool.tile([P, ET, MC], fp8, tag="ot", name="ot")
                for eb in range(ET):
                    pso = ps.tile([P, MC], f32, tag="ps", name="pso")
                    for nt in range(NT // 2):
                        nc.tensor.matmul(
                            pso[:],
                            v_t[:, 2 * nt:2 * nt + 2, eb * P:(eb + 1) * P],
                            e_t[:, 2 * nt:2 * nt + 2, :],
                            start=(nt == 0),
                            stop=(nt == NT // 2 - 1),
                            perf_mode=DR,
                        )
                    nc.vector.tensor_copy(ot_t[:, eb, :], pso[:])
                # output linear partial + deferred softmax normalization
                res4 = opool.tile([P, ET, MC], fp8, tag="res", name="res")
                for ob in range(ET):
                    psl = ps.tile([P, MC], f32, tag="ps", name="psl")
                    for eb in range(ET // 2):
                        nc.tensor.matmul(
                            psl[:],
                            wl_t[:, 2 * eb:2 * eb + 2, ob * P:(ob + 1) * P],
                            ot_t[:, 2 * eb:2 * eb + 2, :],
                            start=(eb == 0),
                            stop=(eb == ET // 2 - 1),
                            perf_mode=DR,
                        )
                    nc.vector.tensor_tensor(
                        res4[:, ob, :], psl[:], rb[:], mybir.AluOpType.mult
                    )
                nc.sync.dma_start(
                    rs_in[p][mc][:].rearrange("(t p) m -> p t m", p=P),
                    res4[:],
                )
                if not single and not no_cc:
                    nc.gpsimd.collective_compute(
                        "ReduceScatter",
                        mybir.AluOpType.add,
                        ins=[rs_in[p][mc].opt()],
                        outs=[rs_out[p][mc].opt()],
                        replica_groups=[list(range(NCORES))],
                    )
                po_queue.append(po_chunk(p, mc))

            def attention(p, feeder_chunks):
                # software pipeline: scores(mc+1) emitted before the output
                # phase of mc, so the PE never waits on the exp drains
                e_cur = scores_phase(p, 0)
                for mc in range(NMC):
                    e_next = scores_phase(p, mc + 1) if mc + 1 < NMC else None
                    if feeder_chunks:
                        for emit in feeder_chunks.pop(0):
                            emit()
                    # flush epilogue work whose RS is ~2 blocks old
                    while len(po_queue) > 1:
                        po_queue.pop(0)()
                    out_phase(p, mc, e_cur)
                    e_cur = e_next

            for emit in proj_units(0):
                emit()
            p1_units = proj_units(1)
            attention(0, [p1_units[i::NMC] for i in range(NMC)])
            attention(1, [])
            while po_queue:
                po_queue.pop(0)()

    nc.compile()
    return nc


_NC_CACHE = {}


def _get_nc():
    if "nc" not in _NC_CACHE:
        _NC_CACHE["nc"] = _build()
    return _NC_CACHE["nc"]


def _make_in_maps(inputs):
    f = np.float32
    b = ml_dtypes.bfloat16
    e4 = ml_dtypes.float8_e4m3

    def c_(x, dt):
        return np.ascontiguousarray(x).astype(dt)

    img = np.asarray(inputs["image_input"], f)
    meta = np.asarray(inputs["metadata_input"], f)
    xt_i = c_(img.T, e4)
    xt_m = c_(meta.T, e4)

    in_maps = []
    for c in range(NCORES):
        m = {
            "xt_i": xt_i,
            "xt_m": xt_m,
            "resid_i": c_(img[:, 64 * c:64 * (c + 1)].T
                          + np.asarray(inputs["b_lin_i"], f)[64 * c:64 * (c + 1)][:, None], f),
            "resid_m": c_(meta[:, 64 * c:64 * (c + 1)].T
                          + np.asarray(inputs["b_lin_m"], f)[64 * c:64 * (c + 1)][:, None], f),
        }
        for p, (Wq, bq_, Wk, bk_, Wv, bv_, Wl) in enumerate([
            (inputs["Wq_m"], inputs["bq_m"], inputs["Wk_i"], inputs["bk_i"],
             inputs["Wv_i"], inputs["bv_i"], inputs["W_lin_i"]),
            (inputs["Wq_i"], inputs["bq_i"], inputs["Wk_m"], inputs["bk_m"],
             inputs["Wv_m"], inputs["bv_m"], inputs["W_lin_m"]),
        ]):
            m[f"wq{p}"] = c_(np.asarray(Wq, f)[c].T * WS, e4)
            m[f"wk{p}"] = c_(np.asarray(Wk, f)[c].T * WS, e4)
            m[f"wv{p}"] = c_(np.asarray(Wv, f)[c].T, e4)
            m[f"wl{p}"] = c_(np.asarray(Wl, f)[:, D * c:D * (c + 1)].T * WS, e4)
            m[f"bq{p}"] = c_(np.asarray(bq_, f)[c].reshape(ET, P).T * WS, f)
            m[f"bv{p}"] = c_(np.asarray(bv_, f)[c][None, :], f)
        in_maps.append(m)
    return in_maps


def _assemble(results):
    out_iT = np.concatenate([results[c]["out_i"] for c in range(NCORES)], axis=0)
    out_mT = np.concatenate([results[c]["out_m"] for c in range(NCORES)], axis=0)
    return np.concatenate([out_iT.T, out_mT.T], axis=1).astype(np.float32)


def kernel(**inputs):
    nc = _get_nc()
    in_maps = _make_in_maps(inputs)
    res = run_bass_kernel_spmd(nc, in_maps, list(range(NCORES)))
    return _assemble(res.results)


if __name__ == "__main__":
    _get_nc()
    print("build ok")

